# revision 1
# baseline (speedup 1.0000x reference)
"""Trainium2 Bass kernel for AdaptiveTemporalAttentionModel.

Data-parallel across the fused B*N item dimension over 8 NeuronCores.
Per-item model: linear(32->64) -> temporal conv(KT=3, 64->64) -> relu ->
last-row attention over T=48 -> MLP 64->256->128->12.

Algebraic restructuring baked in:
 - only the last attention row is needed  => no T x T attention
 - k/v projections eliminated:
     scores[s,b] = h[:,s,b] . (Wk @ q_b) + bk . q_b
     ctx = Wv.T @ (sum_s p_s h_s / Z) + bv
 - input linear + conv fused: W_eff[96,64], with edge-bias corrections
Compute dtype: bf16 (cast during DMA-in), fp32 PSUM/softmax accumulation.
"""

import sys
from contextlib import ExitStack

import numpy as np

sys.path.insert(0, "/opt/trn_rl_repo")

import ml_dtypes  # noqa: E402

import concourse.bass as bass  # noqa: E402
import concourse.tile as tile  # noqa: E402
from concourse import mybir  # noqa: E402

F32 = mybir.dt.float32
BF16 = mybir.dt.bfloat16

B, N, T, E = 16, 2048, 48, 32
C0, C1 = 64, 64
H1, H2 = 256, 128
NPRED = 12
KT = 3
NCORES = 8
ITEMS = B * N            # 32768
IPC = ITEMS // NCORES    # 4096 items per core
TE = T * E               # 1536
NCHUNK = TE // 128       # 12

WSHIFTS = [0, -32, -64, 32, 64, 96]

AF = mybir.ActivationFunctionType
ALU = mybir.AluOpType


def _ap(t_ap, extra_free, offset_elems=0):
    """Build an AP on tile `t_ap`'s tensor: keep its partition dim, replace
    free dims with `extra_free` = [[stride, count], ...] (element strides)."""
    return bass.AP(
        tensor=t_ap.tensor,
        offset=t_ap.offset + offset_elems,
        ap=[list(t_ap.ap[0])] + [list(e) for e in extra_free],
    )


def _split_waits(nc, limits=None):
    """walrus's codegen allows few sync-wait slots per instruction type.
    Move excess waits onto a same-engine no-op right before the
    instruction -- each engine queue executes in program order."""
    if limits is None:
        limits = {}
    for fn in nc.m.functions:
        for blk in fn.blocks:
            out, changed = [], False
            for inst in blk.instructions:
                lim = limits.get(type(inst).__name__, 1)
                si = inst.sync_info
                if lim is not None and si is not None:
                    waits = list(si.on_wait)
                    if len(waits) > lim:
                        for wi, w in enumerate(waits[:-lim]):
                            nop = mybir.InstNoOp(
                                name=f"W{wi}-{inst.name}",
                                text_hint="split_waits")
                            nop.engine = inst.engine
                            nop.sync_info = mybir.SyncInfo(
                                on_wait=[w], on_update=[])
                            out.append(nop)
                        inst.sync_info = mybir.SyncInfo(
                            on_wait=waits[-lim:],
                            on_update=list(si.on_update))
                        changed = True
                out.append(inst)
            if changed:
                blk.instructions = out


def build(n_items=IPC, wb=256, split_waits=True, passes=1):
    """Build the Bass graph for one core processing n_items items."""
    assert n_items % wb == 0 and wb % 128 == 0
    ntiles = n_items // wb
    gpt = wb // 128  # groups (of 128 items) per tile

    nc = bass.Bass()
    x_d = nc.declare_dram_parameter("x", [n_items, TE], BF16, isOutput=False)
    wpack_d = nc.declare_dram_parameter("wpack", [128, 6, C1], BF16, isOutput=False)
    wqsh_d = nc.declare_dram_parameter("wqsh", [128, C1], BF16, isOutput=False)
    wkq_d = nc.declare_dram_parameter("wkq", [C1, 65], BF16, isOutput=False)
    wv_d = nc.declare_dram_parameter("wv", [C1, C1], BF16, isOutput=False)
    w1_d = nc.declare_dram_parameter("w1", [C1, H1], BF16, isOutput=False)
    w2_d = nc.declare_dram_parameter("w2", [H2, H1], BF16, isOutput=False)
    w3_d = nc.declare_dram_parameter("w3", [H2, NPRED], BF16, isOutput=False)
    id_d = nc.declare_dram_parameter("ident", [128, 128], BF16, isOutput=False)
    bconv_d = nc.declare_dram_parameter("bconv", [128, 3], F32, isOutput=False)
    bq_d = nc.declare_dram_parameter("bq", [C1, 1], F32, isOutput=False)
    bv_d = nc.declare_dram_parameter("bv", [C1, 1], F32, isOutput=False)
    b1_d = nc.declare_dram_parameter("b1", [128, 2], F32, isOutput=False)
    b2_d = nc.declare_dram_parameter("b2", [H2, 1], F32, isOutput=False)
    b3_d = nc.declare_dram_parameter("b3", [NPRED, 1], F32, isOutput=False)
    out_d = nc.declare_dram_parameter("out", [NPRED, n_items], F32, isOutput=True)

    with tile.TileContext(nc) as tc, ExitStack() as ctx:
        singles = ctx.enter_context(tc.tile_pool(name="singles", bufs=1))
        xtpool = ctx.enter_context(tc.tile_pool(name="xtpool", bufs=2))
        hpool = ctx.enter_context(tc.tile_pool(name="hpool", bufs=2))
        hipool = ctx.enter_context(tc.tile_pool(name="hipool", bufs=2))
        scrpool = ctx.enter_context(tc.tile_pool(name="scrpool", bufs=5))
        smpool = ctx.enter_context(tc.tile_pool(name="smpool", bufs=6))
        mlpool = ctx.enter_context(tc.tile_pool(name="mlpool", bufs=3))
        psA = ctx.enter_context(tc.tile_pool(name="psA", bufs=4, space="PSUM"))
        psB = ctx.enter_context(tc.tile_pool(name="psB", bufs=4, space="PSUM"))

        # --- load weights (once) ---
        wpack = singles.tile([128, 6, C1], BF16)
        nc.sync.dma_start(out=wpack[:], in_=wpack_d[:])
        wqsh = singles.tile([128, C1], BF16)
        nc.sync.dma_start(out=wqsh[:], in_=wqsh_d[:])
        wkq = singles.tile([C1, 65], BF16)
        nc.sync.dma_start(out=wkq[:], in_=wkq_d[:])
        wv_w = singles.tile([C1, C1], BF16)
        nc.sync.dma_start(out=wv_w[:], in_=wv_d[:])
        w1_w = singles.tile([C1, H1], BF16)
        nc.sync.dma_start(out=w1_w[:], in_=w1_d[:])
        w2_w = singles.tile([H2, H1], BF16)
        nc.sync.dma_start(out=w2_w[:], in_=w2_d[:])
        w3_w = singles.tile([H2, NPRED], BF16)
        nc.sync.dma_start(out=w3_w[:], in_=w3_d[:])
        ident = singles.tile([128, 128], BF16)
        nc.sync.dma_start(out=ident[:], in_=id_d[:])
        bconv = singles.tile([128, 3], F32)
        nc.sync.dma_start(out=bconv[:], in_=bconv_d[:])
        bq_b = singles.tile([C1, 1], F32)
        nc.sync.dma_start(out=bq_b[:], in_=bq_d[:])
        bv_b = singles.tile([C1, 1], F32)
        nc.sync.dma_start(out=bv_b[:], in_=bv_d[:])
        b1_b = singles.tile([128, 2], F32)
        nc.sync.dma_start(out=b1_b[:], in_=b1_d[:])
        b2_b = singles.tile([H2, 1], F32)
        nc.sync.dma_start(out=b2_b[:], in_=b2_d[:])
        b3_b = singles.tile([NPRED, 1], F32)
        nc.sync.dma_start(out=b3_b[:], in_=b3_d[:])

        # conv matmul plan per timestep t: list of (chunk, plo, phi, wlo, whi)
        # contraction over window te in [(t-1)*32, (t+2)*32) clipped to [0, TE)
        def win_pieces(t):
            lo, hi = (t - 1) * E, (t + 2) * E
            wlo = 0
            if lo < 0:
                wlo, lo = E, 0
            if hi > TE:
                hi = TE
            pieces = []
            while lo < hi:
                ch = lo // 128
                plo = lo - ch * 128
                phi = min(hi - ch * 128, 128)
                # PE row-group constraint: pieces must fit a 32/64/128 tile
                # at a 32-aligned base partition (base 32 only allowed if <=32,
                # base 64 if <=64 wide, etc.)
                maxw = 128 - plo if plo == 0 else (32 if plo in (32, 96) else 64)
                phi = min(phi, plo + maxw)
                w = phi - plo
                pieces.append((ch, plo, phi, wlo, wlo + w))
                wlo += w
                lo += w
            return pieces

        def stage_front(it):
            """load+transpose x, conv, q/wq, scores, exp, issue G on gpsimd."""
            # ---- transposing load of x ----
            xt = xtpool.tile([128, NCHUNK, gpt, 128], BF16)
            for g in range(gpt):
                row0 = it * wb + g * 128
                nc.sync.dma_start_transpose(
                    xt[:, :, g, :], x_d[row0 : row0 + 128, :]
                )

            # ---- conv -> h_fm [128=(c,t%2), g, jpair, b] ----
            # jp=23 first so the q/wq chain can start while conv continues
            h_fm = hpool.tile([128, gpt, 24, 128], BF16)

            def conv_pair(jp):
                bank = psA.tile([128, gpt, 128], F32, tag="conv")
                for tp in range(2):
                    t = 2 * jp + tp
                    obase = 64 * tp
                    pieces = win_pieces(t)
                    for i, (ch, plo, phi, wlo, whi) in enumerate(pieces):
                        v = WSHIFTS.index(plo - wlo)
                        nc.tensor.matmul(
                            bank[obase : obase + C1, :, :],
                            wpack[plo:phi, v, :],
                            xt[plo:phi, ch, :, :],
                            start=(i == 0),
                            stop=(i == len(pieces) - 1),
                            tile_position=(plo, obase),
                            skip_group_check=True,
                        )
                bcol = 1 if jp == 0 else (2 if jp == 23 else 0)
                if jp % 6 == 3:
                    nc.vector.tensor_scalar(
                        h_fm[:, :, jp, :],
                        bank[:],
                        bconv[:, bcol : bcol + 1],
                        0.0,
                        ALU.add,
                        ALU.max,
                    )
                else:
                    nc.scalar.activation(
                        h_fm[:, :, jp, :],
                        bank[:],
                        AF.Relu,
                        bias=bconv[:, bcol : bcol + 1],
                        scale=1.0,
                    )

            conv_pair(23)

            # ---- q_last, wq, const (depends only on jp=23) ----
            q_ps = psB.tile([C1, wb], F32, tag="mm")
            nc.tensor.matmul(
                q_ps[:],
                wqsh[64:128, :],
                h_fm[64:128, :, 23, :],
                start=True,
                stop=True,
                tile_position=(64, 0),
                skip_group_check=True,
            )
            q_sb = smpool.tile([C1, wb], BF16, tag="qsb")
            nc.scalar.activation(
                q_sb[:], q_ps[:], AF.Identity, bias=bq_b[:, 0:1], scale=1.0
            )
            wqc_ps = psB.tile([65, wb], F32, tag="mm")
            nc.tensor.matmul(wqc_ps[:], wkq[:], q_sb[:], start=True, stop=True)
            wqc_sb = smpool.tile([65, wb], BF16, tag="wqc")
            nc.vector.tensor_copy(wqc_sb[:], wqc_ps[:])

            for jp in range(23):
                conv_pair(jp)

            wq_im = smpool.tile([128, 2, C1], BF16, tag="wqim")
            const8 = smpool.tile([128, 2], F32, tag="c8")
            h_im = hipool.tile([128, 2, 24, 128], BF16)
            for gg in range(2):
                wqT_ps = psB.tile([128, 65], BF16, tag="mm")
                nc.tensor.transpose(
                    wqT_ps[:], wqc_sb[:, gg * 128 : (gg + 1) * 128],
                    ident[0:65, 0:65]
                )
                nc.vector.tensor_copy(wq_im[:, gg, :], wqT_ps[:, 0:C1])
                nc.scalar.mul(
                    const8[:, gg : gg + 1], wqT_ps[:, C1 : C1 + 1], 0.125
                )
                nc.sync.dma_start_transpose(
                    h_im[:, gg, :, :], h_fm[:, gg, :, :]
                )

            # P = h_im * wq  [128, 2, 24, 2, 64]
            P = scrpool.tile([128, 2, 24, 2, C1], BF16, tag="scr")
            h_v = h_im[:].rearrange("p g j (t c) -> p g j t c", t=2)
            wq_b = _ap(wq_im[:], [[C1, 2], [0, 48], [1, C1]])
            nc.vector.tensor_mul(P[:], h_v, wq_b)
            T1 = scrpool.tile([128, 2, 24, 2, 32], BF16, tag="scrT1")
            nc.vector.tensor_add(T1[:], P[:, :, :, :, 0:32], P[:, :, :, :, 32:64])
            return dict(T1=T1, const8=const8, h_v=h_v)

        def stage_tail(it, st):
            """rest of the scores chain + exp + issue G/U1 on gpsimd."""
            T1, const8, h_v = st["T1"], st["const8"], st["h_v"]
            T2 = scrpool.tile([128, 2, 24, 2, 16], BF16, tag="scrT2")
            nc.vector.tensor_add(T2[:], T1[:, :, :, :, 0:16], T1[:, :, :, :, 16:32])
            T3 = scrpool.tile([128, 2, 24, 2, 8], BF16, tag="scrT3")
            nc.vector.tensor_add(T3[:], T2[:, :, :, :, 0:8], T2[:, :, :, :, 8:16])
            scores = smpool.tile([128, 2, 48], F32, tag="sco")
            nc.vector.tensor_reduce(
                scores[:].rearrange("p g (j t) -> p g j t", j=24),
                T3[:],
                axis=mybir.AxisListType.X,
                op=ALU.add,
            )
            p_exp = smpool.tile([128, 2, 48], BF16, tag="pexp")
            zsum = smpool.tile([128, 2], F32, tag="z")
            for gg in range(2):
                nc.scalar.activation(
                    p_exp[:, gg, :],
                    scores[:, gg, :],
                    AF.Exp,
                    bias=const8[:, gg : gg + 1],
                    scale=0.125,
                    accum_out=zsum[:, gg : gg + 1],
                )
            # G = h_im * p and first g-tree level, both on gpsimd
            G = scrpool.tile([128, 2, 24, 2, C1], BF16, tag="scr")
            p_b = _ap(p_exp[:], [[48, 2], [1, 48], [0, C1]])
            nc.gpsimd.tensor_mul(G[:], h_v, p_b)
            return dict(G=G, zsum=zsum)
            return dict(G=G, rz=rz)

        def stage_back(it, st):
            """g tree-reduce, normalize, transpose, MLP, store."""
            G, zsum = st["G"], st["zsum"]
            rz = smpool.tile([128, 2], F32, tag="rz")
            nc.vector.reciprocal(rz[:], zsum[:])
            U1 = scrpool.tile([128, 2, 12, 2, C1], BF16, tag="scr")
            nc.vector.tensor_add(
                U1[:], G[:, :, 0:12, :, :], G[:, :, 12:24, :, :]
            )
            U2 = scrpool.tile([128, 2, 6, 2, C1], BF16, tag="scrT2")
            nc.vector.tensor_add(
                U2[:], U1[:, :, 0:6, :, :], U1[:, :, 6:12, :, :]
            )
            U3 = scrpool.tile([128, 2, 3, 2, C1], BF16, tag="scrT3")
            nc.vector.tensor_add(
                U3[:], U2[:, :, 0:3, :, :], U2[:, :, 3:6, :, :]
            )
            g128 = smpool.tile([128, 2, 128], F32, tag="g128")
            nc.vector.tensor_reduce(
                g128[:],
                _ap(U3[:], [[3 * 128, 2], [1, 128], [128, 3]]),
                axis=mybir.AxisListType.X,
                op=ALU.add,
            )
            gf = smpool.tile([128, 2, C1], F32, tag="gf")
            nc.vector.tensor_add(
                gf[:], g128[:, :, 0:C1], g128[:, :, C1:128]
            )
            g_sb = smpool.tile([128, 2, C1], BF16, tag="gsb")
            rz_b = _ap(rz[:], [[1, 2], [0, C1]])
            nc.vector.tensor_mul(g_sb[:], gf[:], rz_b)

            gT_sb = mlpool.tile([C1, wb], BF16, tag="gT")
            for gg in range(2):
                gT_ps = psB.tile([C1, 128], BF16, tag="mm")
                nc.tensor.transpose(gT_ps[:], g_sb[:, gg, :], ident[:])
                nc.vector.tensor_copy(
                    gT_sb[:, gg * 128 : (gg + 1) * 128], gT_ps[:]
                )

            # ---- MLP ----
            ctx_ps = psB.tile([C1, wb], F32, tag="mm")
            nc.tensor.matmul(ctx_ps[:], wv_w[:], gT_sb[:], start=True, stop=True)
            ctx_sb = mlpool.tile([C1, wb], BF16, tag="ctx")
            nc.scalar.activation(
                ctx_sb[:], ctx_ps[:], AF.Identity, bias=bv_b[:, 0:1], scale=1.0
            )
            z1_sb = mlpool.tile([128, 2, wb], BF16, tag="z1")
            for half in range(2):
                z1_ps = psB.tile([128, wb], F32, tag="mm")
                nc.tensor.matmul(
                    z1_ps[:],
                    w1_w[:, half * 128 : (half + 1) * 128],
                    ctx_sb[:],
                    start=True,
                    stop=True,
                )
                nc.scalar.activation(
                    z1_sb[:, half, :],
                    z1_ps[:],
                    AF.Relu,
                    bias=b1_b[:, half : half + 1],
                    scale=1.0,
                )
            z2_ps = psB.tile([H2, wb], F32, tag="mm")
            nc.tensor.matmul(
                z2_ps[:], w2_w[:, 0:H2], z1_sb[:, 0, :], start=True, stop=False
            )
            nc.tensor.matmul(
                z2_ps[:], w2_w[:, H2:H1], z1_sb[:, 1, :], start=False, stop=True
            )
            z2_sb = mlpool.tile([H2, wb], BF16, tag="z2")
            nc.scalar.activation(
                z2_sb[:], z2_ps[:], AF.Relu, bias=b2_b[:, 0:1], scale=1.0
            )
            z3_ps = psB.tile([NPRED, wb], F32, tag="mm")
            nc.tensor.matmul(z3_ps[:], w3_w[:], z2_sb[:], start=True, stop=True)
            pred_t = mlpool.tile([NPRED, wb], F32, tag="pred")
            nc.scalar.activation(
                pred_t[:], z3_ps[:], AF.Identity, bias=b3_b[:, 0:1], scale=1.0
            )
            nc.sync.dma_start(
                out=out_d[:, it * wb : (it + 1) * wb], in_=pred_t[:]
            )

        prev = None
        for rep in range(passes):
            for it in range(ntiles):
                fst = stage_front(it)
                if prev is not None:
                    stage_back(prev["it"], prev)
                prev = stage_tail(it, fst)
                prev["it"] = it
        stage_back(prev["it"], prev)

    if split_waits:
        _split_waits(nc)
    return nc


def prep_weights(W_in, b_in, W_conv, b_conv, Wq, bq, Wk, bk, Wv, bv,
                 W1, b1, W2, b2, W3, b3):
    """Host-side weight preprocessing -> dict of small parameter arrays."""
    bf = ml_dtypes.bfloat16
    W_eff = np.concatenate(
        [W_in @ W_conv[j] for j in range(KT)], axis=0
    )  # [96, 64]
    b_eff = b_in @ (W_conv[0] + W_conv[1] + W_conv[2]) + b_conv
    b_left = b_in @ (W_conv[1] + W_conv[2]) + b_conv   # t=0 (no tap 0)
    b_right = b_in @ (W_conv[0] + W_conv[1]) + b_conv  # t=T-1 (no tap 2)
    bconv = np.stack(
        [
            np.concatenate([b_eff, b_eff]),
            np.concatenate([b_left, b_eff]),
            np.concatenate([b_eff, b_right]),
        ],
        axis=1,
    ).astype(np.float32)  # [128, 3]
    wkq = np.concatenate([Wk.T, bk[:, None]], axis=1)  # [64, 65]
    w2 = np.concatenate([W2[0:H2], W2[H2:H1]], axis=1)  # [128, 256]
    shifts = [0, -32, -64, 32, 64, 96]
    wpack = np.zeros((128, 6, C1), np.float32)
    for v, s in enumerate(shifts):
        for p in range(128):
            r = p - s
            if 0 <= r < 96:
                wpack[p, v] = W_eff[r]
    wqsh = np.zeros((128, C1), np.float32)
    wqsh[64:128] = Wq
    return {
        "wpack": np.ascontiguousarray(wpack.astype(bf)),
        "wqsh": np.ascontiguousarray(wqsh.astype(bf)),
        "wkq": np.ascontiguousarray(wkq.astype(bf)),
        "wv": np.ascontiguousarray(Wv.astype(bf)),
        "w1": np.ascontiguousarray(W1.astype(bf)),
        "w2": np.ascontiguousarray(w2.astype(bf)),
        "w3": np.ascontiguousarray(W3.astype(bf)),
        "ident": np.eye(128, dtype=bf),
        "bconv": bconv,
        "bq": np.ascontiguousarray(bq.reshape(C1, 1).astype(np.float32)),
        "bv": np.ascontiguousarray(bv.reshape(C1, 1).astype(np.float32)),
        "b1": np.ascontiguousarray(b1.reshape(2, 128).T.astype(np.float32)),
        "b2": np.ascontiguousarray(b2.reshape(H2, 1).astype(np.float32)),
        "b3": np.ascontiguousarray(b3.reshape(NPRED, 1).astype(np.float32)),
    }


_CACHED = {}


def kernel(x, W_in, b_in, W_conv, b_conv, Wq, bq, Wk, bk, Wv, bv,
           W1, b1, W2, b2, W3, b3, _trace=False):
    from concourse.bass_utils import run_bass_kernel_spmd

    wdict = prep_weights(
        np.asarray(W_in), np.asarray(b_in), np.asarray(W_conv),
        np.asarray(b_conv), np.asarray(Wq), np.asarray(bq), np.asarray(Wk),
        np.asarray(bk), np.asarray(Wv), np.asarray(bv), np.asarray(W1),
        np.asarray(b1), np.asarray(W2), np.asarray(b2), np.asarray(W3),
        np.asarray(b3),
    )
    x = np.asarray(x, dtype=np.float32)
    Bs, Ns, Ts, Es = x.shape
    x_flat = np.ascontiguousarray(
        x.reshape(Bs * Ns, Ts * Es).astype(ml_dtypes.bfloat16)
    )

    if "nc" not in _CACHED:
        _CACHED["nc"] = build()
    nc = _CACHED["nc"]

    in_maps = []
    for c in range(NCORES):
        m = {"x": x_flat[c * IPC : (c + 1) * IPC]}
        m.update(wdict)
        in_maps.append(m)

    res = run_bass_kernel_spmd(
        nc, in_maps, core_ids=list(range(NCORES)), trace=_trace
    )
    outs = [res.results[c]["out"] for c in range(NCORES)]  # each [12, 4096]
    pred = np.concatenate(outs, axis=1)  # [12, 32768]
    out = pred.reshape(NPRED, Bs, Ns).transpose(1, 0, 2)[:, None]  # [B,1,P,N]
    out = np.ascontiguousarray(out.astype(np.float32))
    if _trace:
        return out, res
    return out

